# revision 17
# baseline (speedup 1.0000x reference)
"""Causal single-head attention (B=8, T=4096, C=1024, H=128) on 8 TRN2
NeuronCores, data-parallel over batch: core b computes batch element b.

Host pre-transposes and pre-casts: each core gets xT [C, T] fp16 (so the
contraction dim is already on partitions -- no on-chip transposes) plus
Wq/Wk/Wv [C, H] fp16 replicated. Output is [T, H] f32.

v2 schedule: the kernel is organized around the ScalarE exp spine (the
72 exp calls of ~1.1us each form the serial critical chain
score->exp->AV).  Instead of per-chunk phases, score pairs are emitted
as one flat pipeline paced at the ScalarE rate, with a FIFO queue of PE
"filler" quanta (projection blocks, AV accumulation sub-groups) drained
between pair emissions, so the PE always has dense work interleaved
between score matmuls and ScalarE never waits at chunk boundaries.
Startup: dependency-free PE-warmup matmuls run during the initial DMA
wait (HAM un-throttle), the first weight/x slices use >=512B DMA lines
split across both HWDGE queues, and chunk 0 computes q/k projections
then its score pairs immediately (v-proj/chunk-1 work deferred into the
filler queue) so the first exp issues ~14us earlier than a phase-ordered
schedule.  Diagonal-pair exps cover only the causally needed column
ranges (the junk left of the diagonal is never read; masking of the
diagonal blocks runs on the otherwise-idle GpSimd engine).
"""
import numpy as np

import concourse.bass as bass
import concourse.mybir as mybir
import concourse.tile as tile
from concourse.bass import ts
from contextlib import ExitStack
from collections import deque

F16 = mybir.dt.float16
F32 = mybir.dt.float32

B, T, C, H = 8, 4096, 1024, 128

# ---------------------------------------------------------------------------
# Workaround for the walrus build in this container: each TPB instruction may
# carry at most ONE sync-wait ("Too many sync wait commands" otherwise), but
# Tile attaches several. Keep only the last wait per instruction and hoist the
# others onto preceding same-engine NoOps (engines execute their stream in
# order, so the gating semantics are identical). The tail drain gets the same
# treatment.
# ---------------------------------------------------------------------------
_MAX_WAITS = 1
_orig_add_instruction = tile.TileContext._add_instruction


def _split_waits_add_instruction(self, inst):
    si = inst.sync_info
    if (
        si is not None
        and len(si.on_wait) > _MAX_WAITS
        and inst.engine != mybir.EngineType.Unassigned
    ):
        waits = list(si.on_wait)
        extra, keep = waits[:-_MAX_WAITS], waits[-_MAX_WAITS:]
        for w in extra:
            nop = mybir.InstNoOp(
                name=self.nc.get_next_instruction_name(),
                engine=inst.engine,
                ins=[],
                outs=[],
                bass_nofuse=True,
                sync_info=mybir.SyncInfo(on_wait=[w], on_update=[]),
                debug=inst.debug,
            )
            _orig_add_instruction(self, nop)
        inst.sync_info = mybir.SyncInfo(on_wait=keep, on_update=list(si.on_update))
    return _orig_add_instruction(self, inst)


def _split_drain_and_barrier(self, tick_clock, wait_clock):
    nc = self.nc
    probe = nc.sync.nop(nofuse=True, hint="tile_drain_wait_split")
    wait_clock.add_sem_waits(
        probe.ins, tile.ScopedClock({None: tick_clock.global_clock})
    )
    si = probe.ins.sync_info
    waits = list(si.on_wait) if si is not None else []
    if len(waits) > _MAX_WAITS:
        probe.ins.sync_info = mybir.SyncInfo(
            on_wait=waits[:_MAX_WAITS], on_update=list(si.on_update)
        )
        rest = waits[_MAX_WAITS:]
        for i in range(0, len(rest), _MAX_WAITS):
            extra = nc.sync.nop(nofuse=True, hint=f"tile_drain_wait_split_{i}")
            extra.ins.sync_info = mybir.SyncInfo(
                on_wait=rest[i : i + _MAX_WAITS], on_update=[]
            )
    nc.sync.drain()
    nc.all_engine_barrier()
    assert self.sems is not None
    popped = nc._tile_sem_poison_stack.pop()
    assert popped is self._sem_poison
    nc.clear_and_free_semaphores(list(self.sems.allocated().values()))
    nc.all_engine_barrier()


def _apply_tile_patch():
    tile.TileContext._drain_and_barrier = _split_drain_and_barrier
    tile.TileContext._add_instruction = _split_waits_add_instruction


# ---------------------------------------------------------------------------
# Kernel builder
# ---------------------------------------------------------------------------
def build_attention(dtype=F16):
    TB = T // 128
    CB = C // 128
    NCH = T // 512
    scale = float(H) ** -0.5

    nc = bass.Bass()
    # host pre-arranged layouts: every DMA line is contiguous per partition
    # xt[c_chunk, ci, cb*512 + t] = x[t_global, cb*128 + ci]
    xt = nc.dram_tensor("xt", [NCH, 128, CB * 512], F16, kind="ExternalInput")
    # w*[ci, cb*H + h] = W[cb*128 + ci, h]
    wq = nc.dram_tensor("wq", [128, CB * H], F16, kind="ExternalInput")
    wk = nc.dram_tensor("wk", [128, CB * H], F16, kind="ExternalInput")
    wv = nc.dram_tensor("wv", [128, CB * H], F16, kind="ExternalInput")
    out = nc.dram_tensor("out", [T, H], F32, kind="ExternalOutput")

    with tile.TileContext(nc) as tc, ExitStack() as ctx:
        const = ctx.enter_context(tc.tile_pool(name="const", bufs=1))
        xsb = ctx.enter_context(tc.tile_pool(name="xsb", bufs=3))
        persist = ctx.enter_context(tc.tile_pool(name="persist", bufs=1))
        pP = ctx.enter_context(tc.tile_pool(name="pP", bufs=32))
        osb = ctx.enter_context(tc.tile_pool(name="osb", bufs=4))
        # Dedicated PSUM rings: scores get their own 2-deep ring of 2-bank
        # tiles so a projection/v-proj allocation can never shrink the score
        # pipeline's lookahead (with a shared ring, interleaved proj
        # allocations serialize exp(p) -> scores(p+1) -> exp(p+1) and the
        # exp spine stretches ~1us per pair).  Projections get a 1-deep
        # 2-bank ring; AV accumulators take the remaining 2 banks (8 total).
        pps = ctx.enter_context(tc.tile_pool(name="pps", bufs=2, space="PSUM"))
        ppj = ctx.enter_context(tc.tile_pool(name="ppj", bufs=1, space="PSUM"))
        po = ctx.enter_context(tc.tile_pool(name="po", bufs=1, space="PSUM"))

        # --- PE warmup: the first input DMA cannot complete before ~12us
        # (engine preamble + barrier ~7us, then ~5us trigger-to-completion),
        # so the TensorE would sit idle >3.4us and the HAM clock gate would
        # hold the first ~16 real matmuls at 1.2GHz (607ns each for N=512).
        # ~20 junk N=512 matmuls span the wait and hand the real projection
        # a warm (2.4GHz) array.
        warm16 = const.tile([128, 128], dtype, tag="warm16")
        warmS = const.tile([128, 512], dtype, tag="warmS")
        nc.gpsimd.memset(warm16[:], 0.0)
        nc.gpsimd.memset(warmS[:], 0.0)
        wps = ppj.tile([128, 1024], F32, tag="pj", name="warmup_ps")
        for i in range(12):
            nc.tensor.matmul(
                wps[:, (i % 2) * 512 : (i % 2) * 512 + 512],
                warm16[:],
                warmS[:],
                start=True, stop=True,
            )

        # --- weights on the Scalar HWDGE queue, ordered with chunk-0's
        # upper-half x slices by when the interleaved q/k accumulation
        # consumes them.  First slices of wq/wk cover 2 cb blocks (512B
        # per-partition lines -- at the >=512B threshold below which SDMA
        # falls into read-modify-write) so the first projection matmul can
        # start ASAP.
        wsrc = {"q": wq, "k": wk, "v": wv}
        w16 = {}
        for name in ("q", "k", "v"):
            w16[name] = const.tile(
                [128, CB, H], dtype, tag=f"w{name}", name=f"w16{name}"
            )

        def load_w(name, lo, hi):
            nc.scalar.dma_start(
                w16[name][:, lo:hi, :],
                wsrc[name][:, lo * H : hi * H].rearrange(
                    "ci (cb h) -> ci cb h", cb=hi - lo
                ),
            )

        load_w("q", 0, 2)
        load_w("k", 0, 2)
        load_w("q", 2, CB)
        load_w("k", 2, CB)

        # mask16[jl, ql] = 1 if ql >= jl else 0 (transposed-score layout)
        mask16 = const.tile([128, 128], dtype, tag="mask")
        nc.gpsimd.memset(mask16[:], 1.0)
        nc.gpsimd.affine_select(
            out=mask16[:], in_=mask16[:],
            compare_op=mybir.AluOpType.is_ge,
            fill=0.0, base=0, pattern=[[1, 128]], channel_multiplier=-1,
        )

        qT16 = persist.tile([128, T], dtype, tag="qT")
        kT16 = persist.tile([128, T], dtype, tag="kT")
        v16 = persist.tile([128, TB, H + 1], dtype, tag="v")
        nc.gpsimd.memset(v16[:, :, H : H + 1], 1.0)  # ones column -> denominators

        x16s = {}

        def load_chunk(c):
            x16 = xsb.tile([128, CB, 512], dtype, tag="x16", name=f"x16_c{c}")
            x16s[c] = x16

            def dma(eng, lo, hi):
                eng.dma_start(
                    x16[:, lo:hi, :],
                    xt[c, :, lo * 512 : hi * 512].rearrange(
                        "ci (cb t) -> ci cb t", cb=hi - lo
                    ),
                )
            if c == 0:
                # small first slices, split across the two HWDGE queues, so
                # the first projection matmuls start as early as possible
                dma(nc.sync, 0, 1)
                dma(nc.sync, 1, 2)
                dma(nc.sync, 2, 4)
                dma(nc.scalar, 4, 6)
                dma(nc.scalar, 6, 8)
            elif c == 1:
                dma(nc.sync, 0, 2)
                dma(nc.sync, 2, 4)
                dma(nc.scalar, 4, 8)
            else:
                dma(nc.sync, 0, CB)

        # ------------------------------------------------------------------
        # Work quanta.  Each filler item is (te_cost_ns, emit_fn, born_chunk)
        # and the queue drains FIFO between score-pair emissions.  Every
        # quantum is emission-atomic (allocates and finishes its own PSUM
        # accumulation) so interleaving quanta never splits an open
        # accumulation group across other PSUM-pool allocations.
        # ------------------------------------------------------------------
        # Projection sub-quanta: the dedicated ppj ring means only proj
        # quanta allocate from it, so an accumulation group may stay open
        # across interleaved pair/AV emissions (which touch only pps/po) and
        # projections can be emitted in ~0.5-0.9us pieces -- small enough
        # that the exp stream (buffered by one banked pair) never starves
        # while the PE grinds through filler.
        pjas = {}

        def qk_proj_quantum(c, name, half, lo, hi):
            def emit():
                if c not in pjas:
                    pjas[c] = ppj.tile(
                        [128, 1024], F32, tag="pj", name=f"pjqk_{c}"
                    )
                pja = pjas[c]
                x16 = x16s[c]
                for cb in range(lo, hi):
                    nc.tensor.matmul(
                        pja[:, half * 512 : half * 512 + 512],
                        w16[name][:, cb, :], x16[:, cb, :],
                        start=(cb == 0), stop=(cb == CB - 1),
                    )
                if hi == CB:
                    dst = qT16 if name == "q" else kT16
                    nc.vector.tensor_copy(
                        dst[:, c * 512 : c * 512 + 512],
                        pja[:, half * 512 : half * 512 + 512],
                    )
            return ((hi - lo) * 216 + 30, emit)

        def push_qk_proj(born, c):
            for name, half in (("q", 0), ("k", 1)):
                push(born, qk_proj_quantum(c, name, half, 0, 4))
                push(born, qk_proj_quantum(c, name, half, 4, CB))

        def qk_proj_interleaved(c):
            # chunk-0 startup path: q and k accumulate into the two halves
            # (separate banks) with the cb loop outermost, matching the DMA
            # arrival order of weight/x slices so nothing waits long.
            pja = ppj.tile([128, 1024], F32, tag="pj", name=f"pjqk_{c}")
            x16 = x16s[c]
            for cb in range(CB):
                for name, half in (("q", 0), ("k", 1)):
                    nc.tensor.matmul(
                        pja[:, half * 512 : half * 512 + 512],
                        w16[name][:, cb, :], x16[:, cb, :],
                        start=(cb == 0), stop=(cb == CB - 1),
                    )
            nc.vector.tensor_copy(qT16[:, c * 512 : c * 512 + 512], pja[:, 0:512])
            nc.vector.tensor_copy(
                kT16[:, c * 512 : c * 512 + 512], pja[:, 512:1024]
            )

        def v_proj_quantum(c, tb):
            # one token block per quantum (~0.5us)
            def emit():
                pja = ppj.tile([128, 1024], F32, tag="pj", name=f"pjv_{c}_{tb}")
                x16 = x16s[c]
                for cb in range(CB):
                    nc.tensor.matmul(
                        pja[:, tb * 128 : (tb + 1) * 128],
                        x16[:, cb, ts(tb, 128)], w16["v"][:, cb, :],
                        start=(cb == 0), stop=(cb == CB - 1),
                    )
                nc.vector.tensor_copy(
                    v16[:, c * 4 + tb, 0:H],
                    pja[:, tb * 128 : (tb + 1) * 128],
                )
            return (8 * 62, emit)

        def push_v_proj(born, c):
            for tb in range(4):
                push(born, v_proj_quantum(c, tb))

        # AV state: PSUM accumulators per (chunk, group-of-two-query-blocks)
        opss = {}

        def get_ops(c, g):
            if (c, g) not in opss:
                opss[(c, g)] = po.tile(
                    [128, 2, 256], F32, tag=f"o{g}", name=f"op_{c}_{g}"
                )
            return opss[(c, g)]

        p16s = {}  # (c, pair) -> p16 tile

        def av_quantum(c, qb, j_lo, j_hi, i_q):
            # AV accumulation sub-range for query block qb of chunk c,
            # key blocks j_lo..j_hi-1 (full group runs ji 0..i_q).
            def emit():
                ops = get_ops(c, qb // 2)
                for ji in range(j_lo, j_hi):
                    off = (ji % 2) * 512
                    nc.tensor.matmul(
                        ops[:, qb % 2, 0 : H + 1],
                        p16s[(c, ji // 2)][:, off + qb * 128 : off + (qb + 1) * 128],
                        v16[:, ji, :],
                        start=(ji == 0), stop=(ji == i_q),
                    )
                if j_hi == i_q + 1:
                    # group complete: normalize + store (DVE + DMA, no PE)
                    sl = ops[:, qb % 2, :]
                    rec = osb.tile([128, 1], F32, tag="rec")
                    nc.vector.reciprocal(rec[:], sl[:, H : H + 1])
                    o32 = osb.tile([128, H], F32, tag="o32")
                    nc.vector.tensor_scalar_mul(o32[:], sl[:, 0:H], rec[:])
                    nc.sync.dma_start(
                        out[c * 512 + qb * 128 : c * 512 + (qb + 1) * 128, :],
                        o32[:],
                    )
            return (62 * (j_hi - j_lo), emit)

        filler = deque()
        credit = [0.0]

        def push(c_born, quantum):
            filler.append((quantum[0], quantum[1], c_born))

        def av_subs(c, qb):
            # 12-MM sub-quanta with the pair index each is gated on
            i_q = 4 * c + qb
            subs = []
            j = 0
            while j < i_q + 1:
                j2 = min(j + 12, i_q + 1)
                subs.append(((j2 - 1) // 2, av_quantum(c, qb, j, j2, i_q)))
                j = j2
            return subs

        def drain(budget):
            # credit-carrying: a chunky quantum overdraws, later pairs repay
            credit[0] = min(credit[0], 0.0) + budget
            while credit[0] > 0 and filler:
                cost, fn, _ = filler.popleft()
                fn()
                credit[0] -= cost

        def force_drain_older_than(c):
            # everything born 2+ chunks ago must be emitted before chunk c:
            # (a) qk-proj(c) quanta precede chunk-c score pairs, (b) v-proj
            # and AV readers of retiring x16/p16/ops buffer instances are
            # emitted before the instance's next writer allocates it.
            while filler and filler[0][2] <= c - 2:
                _, fn, _ = filler.popleft()
                fn()

        def emit_pair(c, p):
            t0 = c * 512
            last = p == 2 * c + 1
            sp = pps.tile([128, 1024], F32, tag="sp", name=f"sp_{c}_{p}")
            for ji, off in ((2 * p, 0), (2 * p + 1, 512)):
                d = ji - 4 * c
                # cols left of the diagonal are skipped only on the last
                # (d=2,3) pair, whose exp is range-restricted to match; the
                # d=0,1 pair computes full width so its full-tile exp never
                # reads bytes of the previous PSUM-ring instance.
                q_lo = d * 128 if (last and d > 0) else 0
                nc.tensor.matmul(
                    sp[:, off + q_lo : off + 512],
                    kT16[:, ts(ji, 128)],
                    qT16[:, t0 + q_lo : t0 + 512],
                    start=True, stop=True,
                )
            p16 = pP.tile([128, 1024], dtype, tag="p", name=f"p16_{c}_{p}")
            p16s[(c, p)] = p16
            if last:
                # diagonal pair (d=2,3): exp only the causally needed column
                # ranges; junk left of each diagonal block is never read.
                nc.scalar.activation(
                    p16[:, 256:512], sp[:, 256:512],
                    mybir.ActivationFunctionType.Exp, scale=scale,
                )
                nc.scalar.activation(
                    p16[:, 896:1024], sp[:, 896:1024],
                    mybir.ActivationFunctionType.Exp, scale=scale,
                )
            else:
                nc.scalar.activation(
                    p16[:], sp[:],
                    mybir.ActivationFunctionType.Exp, scale=scale,
                )
            for ji, off in ((2 * p, 0), (2 * p + 1, 512)):
                d = ji - 4 * c
                if d >= 0:
                    # triangular mask on the diagonal block (GpSimd: keeps
                    # the DVE cast/normalize queue out of the exp->AV chain)
                    nc.gpsimd.tensor_mul(
                        p16[:, off + d * 128 : off + (d + 1) * 128],
                        p16[:, off + d * 128 : off + (d + 1) * 128],
                        mask16[:],
                    )

        # ------------------------------------------------------------------
        # Flat pipeline
        # ------------------------------------------------------------------
        load_chunk(0)
        load_w("v", 0, CB)
        load_chunk(1)

        # chunk 0 q/k projections emitted directly: the first score pair
        # (and so the first exp) issues as soon as x(0) lands.
        qk_proj_interleaved(0)

        # seed filler: chunk-1 q/k projections, then chunk-0 v projection
        push_qk_proj(-1, 1)
        push_v_proj(-1, 0)

        # Filler-per-pair budget.  Total filler (~73us) exceeds what the
        # spine can host at the minimum rate (~715ns/pair), and the spine
        # has slack vs the PE-bound total, so drain a bit greedily -- the
        # banked pair in the score ring absorbs the jitter.
        PACE = 1050

        for c in range(NCH):
            force_drain_older_than(c)
            if c + 2 < NCH:
                load_chunk(c + 2)
            # av0 (o0 bank) and av2 (o1 bank) spread across this chunk's
            # pairs as their input pairs land; av1/av3 follow their
            # bank-mates completely (same-bank groups must not overlap).
            sub0, sub2 = av_subs(c, 0), av_subs(c, 2)
            i0 = i2 = 0
            p_push = min(2, 2 * c + 1)
            for p in range(2 * c + 2):
                emit_pair(c, p)
                if p == p_push and c + 2 < NCH:
                    push_qk_proj(c, c + 2)
                while i0 < len(sub0) and sub0[i0][0] <= p:
                    push(c, sub0[i0][1]); i0 += 1
                while i2 < len(sub2) and sub2[i2][0] <= p:
                    push(c, sub2[i2][1]); i2 += 1
                if p == 2 * c:
                    for _, q in av_subs(c, 1):
                        push(c, q)
                drain(PACE)
            for _, q in av_subs(c, 3):
                push(c, q)
            if c + 1 < NCH:
                # v-proj lands late on purpose: it is only needed by the
                # final AV sub-quanta of its own chunk, and holding it back
                # keeps the PE fed during the exp-bound late chunks.
                push_v_proj(c, c + 1)
        while filler:
            filler.popleft()[1]()

    return nc


_NC_CACHE = None


def _get_nc():
    global _NC_CACHE
    if _NC_CACHE is None:
        _apply_tile_patch()
        _NC_CACHE = build_attention()
    return _NC_CACHE


def kernel(x, Wk, Wq, Wv, trace=False):
    """Full inputs in, full output out. Shards batch across the 8 cores."""
    from concourse.bass_utils import run_bass_kernel_spmd

    x = np.asarray(x, dtype=np.float32)
    assert x.shape == (B, T, C), x.shape

    def _warr(w):
        # [C, H] f32 -> [ci, cb*H] fp16 so the on-chip tile loads contiguously
        w16 = np.asarray(w, dtype=np.float32).astype(np.float16)
        return np.ascontiguousarray(
            w16.reshape(C // 128, 128, H).transpose(1, 0, 2).reshape(128, -1)
        )

    Wk16, Wq16, Wv16 = _warr(Wk), _warr(Wq), _warr(Wv)
    # [B,T,C] -> xt[b, chunk, ci, cb*512+t] = x[b, chunk*512+t, cb*128+ci]
    xT16 = np.ascontiguousarray(
        x.transpose(0, 2, 1)
        .astype(np.float16)
        .reshape(B, C // 128, 128, T // 512, 512)
        .transpose(0, 3, 2, 1, 4)
        .reshape(B, T // 512, 128, -1)
    )

    nc = _get_nc()
    in_maps = [
        {"xt": xT16[b], "wq": Wq16, "wk": Wk16, "wv": Wv16} for b in range(B)
    ]
    res = run_bass_kernel_spmd(nc, in_maps, core_ids=list(range(B)), trace=trace)
    outp = np.stack([res.results[b]["out"] for b in range(B)], axis=0)
    if trace:
        global _LAST_RES
        _LAST_RES = res
        return outp, res.exec_time_ns
    return outp


# revision 21
# speedup vs baseline: 1.0157x; 1.0157x over previous
"""Causal single-head attention (B=8, T=4096, C=1024, H=128) on 8 TRN2
NeuronCores, data-parallel over batch: core b computes batch element b.

Host pre-transposes and pre-casts: each core gets xT [C, T] fp16 (so the
contraction dim is already on partitions -- no on-chip transposes) plus
Wq/Wk/Wv [C, H] fp16 replicated. Output is [T, H] f32.

v2 schedule: the kernel is organized around the ScalarE exp spine (the
72 exp calls of ~1.1us each form the serial critical chain
score->exp->AV).  Instead of per-chunk phases, score pairs are emitted
as one flat pipeline paced at the ScalarE rate, with a FIFO queue of PE
"filler" quanta (projection blocks, AV accumulation sub-groups) drained
between pair emissions, so the PE always has dense work interleaved
between score matmuls and ScalarE never waits at chunk boundaries.
Startup: dependency-free PE-warmup matmuls run during the initial DMA
wait (HAM un-throttle), the first weight/x slices use >=512B DMA lines
split across both HWDGE queues, and chunk 0 computes q/k projections
then its score pairs immediately (v-proj/chunk-1 work deferred into the
filler queue) so the first exp issues ~14us earlier than a phase-ordered
schedule.  Diagonal-pair exps cover only the causally needed column
ranges (the junk left of the diagonal is never read; masking of the
diagonal blocks runs on the otherwise-idle GpSimd engine).
"""
import numpy as np

import concourse.bass as bass
import concourse.mybir as mybir
import concourse.tile as tile
from concourse.bass import ts
from contextlib import ExitStack
from collections import deque

F16 = mybir.dt.float16
F32 = mybir.dt.float32

B, T, C, H = 8, 4096, 1024, 128

# ---------------------------------------------------------------------------
# Workaround for the walrus build in this container: each TPB instruction may
# carry at most ONE sync-wait ("Too many sync wait commands" otherwise), but
# Tile attaches several. Keep only the last wait per instruction and hoist the
# others onto preceding same-engine NoOps (engines execute their stream in
# order, so the gating semantics are identical). The tail drain gets the same
# treatment.
# ---------------------------------------------------------------------------
_MAX_WAITS = 1
_orig_add_instruction = tile.TileContext._add_instruction


def _split_waits_add_instruction(self, inst):
    si = inst.sync_info
    if (
        si is not None
        and len(si.on_wait) > _MAX_WAITS
        and inst.engine != mybir.EngineType.Unassigned
    ):
        waits = list(si.on_wait)
        extra, keep = waits[:-_MAX_WAITS], waits[-_MAX_WAITS:]
        for w in extra:
            nop = mybir.InstNoOp(
                name=self.nc.get_next_instruction_name(),
                engine=inst.engine,
                ins=[],
                outs=[],
                bass_nofuse=True,
                sync_info=mybir.SyncInfo(on_wait=[w], on_update=[]),
                debug=inst.debug,
            )
            _orig_add_instruction(self, nop)
        inst.sync_info = mybir.SyncInfo(on_wait=keep, on_update=list(si.on_update))
    return _orig_add_instruction(self, inst)


def _split_drain_and_barrier(self, tick_clock, wait_clock):
    nc = self.nc
    probe = nc.sync.nop(nofuse=True, hint="tile_drain_wait_split")
    wait_clock.add_sem_waits(
        probe.ins, tile.ScopedClock({None: tick_clock.global_clock})
    )
    si = probe.ins.sync_info
    waits = list(si.on_wait) if si is not None else []
    if len(waits) > _MAX_WAITS:
        probe.ins.sync_info = mybir.SyncInfo(
            on_wait=waits[:_MAX_WAITS], on_update=list(si.on_update)
        )
        rest = waits[_MAX_WAITS:]
        for i in range(0, len(rest), _MAX_WAITS):
            extra = nc.sync.nop(nofuse=True, hint=f"tile_drain_wait_split_{i}")
            extra.ins.sync_info = mybir.SyncInfo(
                on_wait=rest[i : i + _MAX_WAITS], on_update=[]
            )
    nc.sync.drain()
    nc.all_engine_barrier()
    assert self.sems is not None
    popped = nc._tile_sem_poison_stack.pop()
    assert popped is self._sem_poison
    nc.clear_and_free_semaphores(list(self.sems.allocated().values()))
    nc.all_engine_barrier()


def _apply_tile_patch():
    tile.TileContext._drain_and_barrier = _split_drain_and_barrier
    tile.TileContext._add_instruction = _split_waits_add_instruction


# ---------------------------------------------------------------------------
# Kernel builder
# ---------------------------------------------------------------------------
def build_attention(dtype=F16):
    TB = T // 128
    CB = C // 128
    NCH = T // 512
    scale = float(H) ** -0.5

    nc = bass.Bass()
    # host pre-arranged layouts: every DMA line is contiguous per partition
    # xt[c_chunk, ci, cb*512 + t] = x[t_global, cb*128 + ci]
    xt = nc.dram_tensor("xt", [NCH, 128, CB * 512], F16, kind="ExternalInput")
    # w*[ci, cb*H + h] = W[cb*128 + ci, h]
    wq = nc.dram_tensor("wq", [128, CB * H], F16, kind="ExternalInput")
    wk = nc.dram_tensor("wk", [128, CB * H], F16, kind="ExternalInput")
    wv = nc.dram_tensor("wv", [128, CB * H], F16, kind="ExternalInput")
    out = nc.dram_tensor("out", [T, H], F32, kind="ExternalOutput")

    with tile.TileContext(nc) as tc, ExitStack() as ctx:
        const = ctx.enter_context(tc.tile_pool(name="const", bufs=1))
        xsb = ctx.enter_context(tc.tile_pool(name="xsb", bufs=4))
        persist = ctx.enter_context(tc.tile_pool(name="persist", bufs=1))
        pP = ctx.enter_context(tc.tile_pool(name="pP", bufs=32))
        osb = ctx.enter_context(tc.tile_pool(name="osb", bufs=4))
        # Dedicated PSUM rings: scores get their own 2-deep ring of 2-bank
        # tiles so a projection/v-proj allocation can never shrink the score
        # pipeline's lookahead (with a shared ring, interleaved proj
        # allocations serialize exp(p) -> scores(p+1) -> exp(p+1) and the
        # exp spine stretches ~1us per pair).  Projections get a 1-deep
        # 2-bank ring; AV accumulators take the remaining 2 banks (8 total).
        pps = ctx.enter_context(tc.tile_pool(name="pps", bufs=2, space="PSUM"))
        ppj = ctx.enter_context(tc.tile_pool(name="ppj", bufs=1, space="PSUM"))
        po = ctx.enter_context(tc.tile_pool(name="po", bufs=1, space="PSUM"))

        # --- PE warmup: the first input DMA cannot complete before ~12us
        # (engine preamble + barrier ~7us, then ~5us trigger-to-completion),
        # so the TensorE would sit idle >3.4us and the HAM clock gate would
        # hold the first ~16 real matmuls at 1.2GHz (607ns each for N=512).
        # ~20 junk N=512 matmuls span the wait and hand the real projection
        # a warm (2.4GHz) array.
        warm16 = const.tile([128, 128], dtype, tag="warm16")
        warmS = const.tile([128, 512], dtype, tag="warmS")
        nc.gpsimd.memset(warm16[:], 0.0)
        nc.gpsimd.memset(warmS[:], 0.0)
        wps = ppj.tile([128, 1024], F32, tag="pj", name="warmup_ps")
        for i in range(12):
            nc.tensor.matmul(
                wps[:, (i % 2) * 512 : (i % 2) * 512 + 512],
                warm16[:],
                warmS[:],
                start=True, stop=True,
            )

        # --- weights on the Scalar HWDGE queue, ordered with chunk-0's
        # upper-half x slices by when the interleaved q/k accumulation
        # consumes them.  First slices of wq/wk cover 2 cb blocks (512B
        # per-partition lines -- at the >=512B threshold below which SDMA
        # falls into read-modify-write) so the first projection matmul can
        # start ASAP.
        wsrc = {"q": wq, "k": wk, "v": wv}
        w16 = {}
        for name in ("q", "k", "v"):
            w16[name] = const.tile(
                [128, CB, H], dtype, tag=f"w{name}", name=f"w16{name}"
            )

        def load_w(name, lo, hi):
            nc.scalar.dma_start(
                w16[name][:, lo:hi, :],
                wsrc[name][:, lo * H : hi * H].rearrange(
                    "ci (cb h) -> ci cb h", cb=hi - lo
                ),
            )

        load_w("q", 0, 2)
        load_w("k", 0, 2)
        load_w("q", 2, CB)
        load_w("k", 2, CB)

        # mask16[jl, ql] = 1 if ql >= jl else 0 (transposed-score layout)
        mask16 = const.tile([128, 128], dtype, tag="mask")
        nc.gpsimd.memset(mask16[:], 1.0)
        nc.gpsimd.affine_select(
            out=mask16[:], in_=mask16[:],
            compare_op=mybir.AluOpType.is_ge,
            fill=0.0, base=0, pattern=[[1, 128]], channel_multiplier=-1,
        )

        qT16 = persist.tile([128, T], dtype, tag="qT")
        kT16 = persist.tile([128, T], dtype, tag="kT")
        v16 = persist.tile([128, TB, H + 1], dtype, tag="v")
        nc.gpsimd.memset(v16[:, :, H : H + 1], 1.0)  # ones column -> denominators

        x16s = {}

        def load_chunk(c):
            x16 = xsb.tile([128, CB, 512], dtype, tag="x16", name=f"x16_c{c}")
            x16s[c] = x16

            def dma(eng, lo, hi):
                eng.dma_start(
                    x16[:, lo:hi, :],
                    xt[c, :, lo * 512 : hi * 512].rearrange(
                        "ci (cb t) -> ci cb t", cb=hi - lo
                    ),
                )
            if c == 0:
                # small first slices, split across the two HWDGE queues, so
                # the first projection matmuls start as early as possible
                dma(nc.sync, 0, 1)
                dma(nc.sync, 1, 2)
                dma(nc.sync, 2, 4)
                dma(nc.scalar, 4, 6)
                dma(nc.scalar, 6, 8)
            elif c == 1:
                dma(nc.sync, 0, 2)
                dma(nc.sync, 2, 4)
                dma(nc.scalar, 4, 8)
            else:
                # halves, so the first projection sub-quantum (cb 0-3) can
                # start as soon as the lower half lands
                dma(nc.sync, 0, 4)
                dma(nc.sync, 4, CB)

        # ------------------------------------------------------------------
        # Work quanta.  Each filler item is (te_cost_ns, emit_fn, born_chunk)
        # and the queue drains FIFO between score-pair emissions.  Every
        # quantum is emission-atomic (allocates and finishes its own PSUM
        # accumulation) so interleaving quanta never splits an open
        # accumulation group across other PSUM-pool allocations.
        # ------------------------------------------------------------------
        # Projection sub-quanta: the dedicated ppj ring means only proj
        # quanta allocate from it, so an accumulation group may stay open
        # across interleaved pair/AV emissions (which touch only pps/po) and
        # projections can be emitted in ~0.5-0.9us pieces -- small enough
        # that the exp stream (buffered by one banked pair) never starves
        # while the PE grinds through filler.
        pjas = {}

        def qk_proj_quantum(c, name, half, lo, hi):
            def emit():
                if c not in pjas:
                    pjas[c] = ppj.tile(
                        [128, 1024], F32, tag="pj", name=f"pjqk_{c}"
                    )
                pja = pjas[c]
                x16 = x16s[c]
                for cb in range(lo, hi):
                    nc.tensor.matmul(
                        pja[:, half * 512 : half * 512 + 512],
                        w16[name][:, cb, :], x16[:, cb, :],
                        start=(cb == 0), stop=(cb == CB - 1),
                    )
                if hi == CB:
                    dst = qT16 if name == "q" else kT16
                    nc.vector.tensor_copy(
                        dst[:, c * 512 : c * 512 + 512],
                        pja[:, half * 512 : half * 512 + 512],
                    )
            return ((hi - lo) * 216 + 30, emit)

        def push_qk_proj(born, c):
            for name, half in (("q", 0), ("k", 1)):
                push(born, qk_proj_quantum(c, name, half, 0, 4))
                push(born, qk_proj_quantum(c, name, half, 4, CB))

        def qk_proj_interleaved(c):
            # chunk-0 startup path: q and k accumulate into the two halves
            # (separate banks) with the cb loop outermost, matching the DMA
            # arrival order of weight/x slices so nothing waits long.
            pja = ppj.tile([128, 1024], F32, tag="pj", name=f"pjqk_{c}")
            x16 = x16s[c]
            for cb in range(CB):
                for name, half in (("q", 0), ("k", 1)):
                    nc.tensor.matmul(
                        pja[:, half * 512 : half * 512 + 512],
                        w16[name][:, cb, :], x16[:, cb, :],
                        start=(cb == 0), stop=(cb == CB - 1),
                    )
            nc.vector.tensor_copy(qT16[:, c * 512 : c * 512 + 512], pja[:, 0:512])
            nc.vector.tensor_copy(
                kT16[:, c * 512 : c * 512 + 512], pja[:, 512:1024]
            )

        def v_proj_quantum(c, tb):
            # one token block per quantum (~0.5us)
            def emit():
                pja = ppj.tile([128, 1024], F32, tag="pj", name=f"pjv_{c}_{tb}")
                x16 = x16s[c]
                for cb in range(CB):
                    nc.tensor.matmul(
                        pja[:, tb * 128 : (tb + 1) * 128],
                        x16[:, cb, ts(tb, 128)], w16["v"][:, cb, :],
                        start=(cb == 0), stop=(cb == CB - 1),
                    )
                nc.vector.tensor_copy(
                    v16[:, c * 4 + tb, 0:H],
                    pja[:, tb * 128 : (tb + 1) * 128],
                )
            return (8 * 62, emit)

        def push_v_proj(born, c):
            for tb in range(4):
                push(born, v_proj_quantum(c, tb))

        # AV state: PSUM accumulators per (chunk, group-of-two-query-blocks)
        opss = {}

        def get_ops(c, g):
            if (c, g) not in opss:
                opss[(c, g)] = po.tile(
                    [128, 2, 256], F32, tag=f"o{g}", name=f"op_{c}_{g}"
                )
            return opss[(c, g)]

        p16s = {}  # (c, pair) -> p16 tile

        def av_quantum(c, qb, j_lo, j_hi, i_q):
            # AV accumulation sub-range for query block qb of chunk c,
            # key blocks j_lo..j_hi-1 (full group runs ji 0..i_q).
            def emit():
                ops = get_ops(c, qb // 2)
                for ji in range(j_lo, j_hi):
                    off = (ji % 2) * 512
                    nc.tensor.matmul(
                        ops[:, qb % 2, 0 : H + 1],
                        p16s[(c, ji // 2)][:, off + qb * 128 : off + (qb + 1) * 128],
                        v16[:, ji, :],
                        start=(ji == 0), stop=(ji == i_q),
                    )
                if j_hi == i_q + 1:
                    # group complete: normalize + store.  Output goes out on
                    # the GpSimd SWDGE queue so stores never sit ahead of
                    # x-chunk loads in the Sync HWDGE FIFO.
                    sl = ops[:, qb % 2, :]
                    rec = osb.tile([128, 1], F32, tag="rec")
                    nc.vector.reciprocal(rec[:], sl[:, H : H + 1])
                    o32 = osb.tile([128, H], F32, tag="o32")
                    nc.vector.tensor_scalar_mul(o32[:], sl[:, 0:H], rec[:])
                    nc.gpsimd.dma_start(
                        out[c * 512 + qb * 128 : c * 512 + (qb + 1) * 128, :],
                        o32[:],
                    )
            return (62 * (j_hi - j_lo), emit)

        filler = deque()
        credit = [0.0]

        def push(c_born, quantum):
            filler.append((quantum[0], quantum[1], c_born))

        def av_subs(c, qb):
            # 12-MM sub-quanta with the pair index each is gated on
            i_q = 4 * c + qb
            subs = []
            j = 0
            while j < i_q + 1:
                j2 = min(j + 12, i_q + 1)
                subs.append(((j2 - 1) // 2, av_quantum(c, qb, j, j2, i_q)))
                j = j2
            return subs

        def drain(budget):
            # credit-carrying: a chunky quantum overdraws, later pairs repay
            credit[0] = min(credit[0], 0.0) + budget
            while credit[0] > 0 and filler:
                cost, fn, _ = filler.popleft()
                fn()
                credit[0] -= cost

        def force_drain_older_than(c):
            # everything born 2+ chunks ago must be emitted before chunk c:
            # (a) qk-proj(c) quanta precede chunk-c score pairs, (b) v-proj
            # and AV readers of retiring x16/p16/ops buffer instances are
            # emitted before the instance's next writer allocates it.
            while filler and filler[0][2] <= c - 2:
                _, fn, _ = filler.popleft()
                fn()

        def emit_pair(c, p):
            t0 = c * 512
            last = p == 2 * c + 1
            sp = pps.tile([128, 1024], F32, tag="sp", name=f"sp_{c}_{p}")
            for ji, off in ((2 * p, 0), (2 * p + 1, 512)):
                d = ji - 4 * c
                # cols left of the diagonal are skipped only on the last
                # (d=2,3) pair, whose exp is range-restricted to match; the
                # d=0,1 pair computes full width so its full-tile exp never
                # reads bytes of the previous PSUM-ring instance.
                q_lo = d * 128 if (last and d > 0) else 0
                nc.tensor.matmul(
                    sp[:, off + q_lo : off + 512],
                    kT16[:, ts(ji, 128)],
                    qT16[:, t0 + q_lo : t0 + 512],
                    start=True, stop=True,
                )
            p16 = pP.tile([128, 1024], dtype, tag="p", name=f"p16_{c}_{p}")
            p16s[(c, p)] = p16
            if last:
                # diagonal pair (d=2,3): exp only the causally needed column
                # ranges; junk left of each diagonal block is never read.
                nc.scalar.activation(
                    p16[:, 256:512], sp[:, 256:512],
                    mybir.ActivationFunctionType.Exp, scale=scale,
                )
                nc.scalar.activation(
                    p16[:, 896:1024], sp[:, 896:1024],
                    mybir.ActivationFunctionType.Exp, scale=scale,
                )
            else:
                nc.scalar.activation(
                    p16[:], sp[:],
                    mybir.ActivationFunctionType.Exp, scale=scale,
                )
            for ji, off in ((2 * p, 0), (2 * p + 1, 512)):
                d = ji - 4 * c
                if d >= 0:
                    # triangular mask on the diagonal block (GpSimd: keeps
                    # the DVE cast/normalize queue out of the exp->AV chain)
                    nc.gpsimd.tensor_mul(
                        p16[:, off + d * 128 : off + (d + 1) * 128],
                        p16[:, off + d * 128 : off + (d + 1) * 128],
                        mask16[:],
                    )

        # ------------------------------------------------------------------
        # Flat pipeline
        # ------------------------------------------------------------------
        load_chunk(0)
        load_w("v", 0, CB)
        load_chunk(1)

        # chunk 0 q/k projections emitted directly: the first score pair
        # (and so the first exp) issues as soon as x(0) lands.
        qk_proj_interleaved(0)

        # seed filler: chunk-1 q/k projections, then chunk-0 v projection
        push_qk_proj(-1, 1)
        push_v_proj(-1, 0)

        # Filler-per-pair budget.  Total filler (~73us) exceeds what the
        # spine can host at the minimum rate (~715ns/pair), and the spine
        # has slack vs the PE-bound total, so drain a bit greedily -- the
        # banked pair in the score ring absorbs the jitter.
        PACE = 1050

        load_chunk(2)  # 3-chunk prefetch depth hides the ~7-10us chunk DMA

        for c in range(NCH):
            force_drain_older_than(c)
            if c + 3 < NCH:
                load_chunk(c + 3)
            # av0 (o0 bank) and av2 (o1 bank) spread across this chunk's
            # pairs as their input pairs land; av1/av3 follow their
            # bank-mates completely (same-bank groups must not overlap).
            sub0, sub2 = av_subs(c, 0), av_subs(c, 2)
            i0 = i2 = 0
            p_push = min(c + 1, 2 * c + 1)
            for p in range(2 * c + 2):
                emit_pair(c, p)
                if p == p_push and c + 2 < NCH:
                    push_qk_proj(c, c + 2)
                while i0 < len(sub0) and sub0[i0][0] <= p:
                    push(c, sub0[i0][1]); i0 += 1
                while i2 < len(sub2) and sub2[i2][0] <= p:
                    push(c, sub2[i2][1]); i2 += 1
                if p == 2 * c:
                    for _, q in av_subs(c, 1):
                        push(c, q)
                drain(PACE)
            for _, q in av_subs(c, 3):
                push(c, q)
            if c + 1 < NCH:
                # v-proj lands late on purpose: it is only needed by the
                # final AV sub-quanta of its own chunk, and holding it back
                # keeps the PE fed during the exp-bound late chunks.
                push_v_proj(c, c + 1)
        while filler:
            filler.popleft()[1]()

    return nc


_NC_CACHE = None


def _get_nc():
    global _NC_CACHE
    if _NC_CACHE is None:
        _apply_tile_patch()
        _NC_CACHE = build_attention()
    return _NC_CACHE


def kernel(x, Wk, Wq, Wv, trace=False):
    """Full inputs in, full output out. Shards batch across the 8 cores."""
    from concourse.bass_utils import run_bass_kernel_spmd

    x = np.asarray(x, dtype=np.float32)
    assert x.shape == (B, T, C), x.shape

    def _warr(w):
        # [C, H] f32 -> [ci, cb*H] fp16 so the on-chip tile loads contiguously
        w16 = np.asarray(w, dtype=np.float32).astype(np.float16)
        return np.ascontiguousarray(
            w16.reshape(C // 128, 128, H).transpose(1, 0, 2).reshape(128, -1)
        )

    Wk16, Wq16, Wv16 = _warr(Wk), _warr(Wq), _warr(Wv)
    # [B,T,C] -> xt[b, chunk, ci, cb*512+t] = x[b, chunk*512+t, cb*128+ci]
    xT16 = np.ascontiguousarray(
        x.transpose(0, 2, 1)
        .astype(np.float16)
        .reshape(B, C // 128, 128, T // 512, 512)
        .transpose(0, 3, 2, 1, 4)
        .reshape(B, T // 512, 128, -1)
    )

    nc = _get_nc()
    in_maps = [
        {"xt": xT16[b], "wq": Wq16, "wk": Wk16, "wv": Wv16} for b in range(B)
    ]
    res = run_bass_kernel_spmd(nc, in_maps, core_ids=list(range(B)), trace=trace)
    outp = np.stack([res.results[b]["out"] for b in range(B)], axis=0)
    if trace:
        global _LAST_RES
        _LAST_RES = res
        return outp, res.exec_time_ns
    return outp
